# revision 4
# baseline (speedup 1.0000x reference)
"""Trainium2 Bass kernel for MQA cross-attention (nn_CrossAttention).

Reference computation (fp32):
    q = (x @ Wq).reshape(b, n, 16, 128).transpose(0,2,1,3) * 128**-0.5
    sim = q @ k^T   (k/v shared across heads, MQA)
    out = softmax(sim) @ v
    y = out.merge_heads @ Wo

Sharding: pure sequence-parallel across 8 cores. Each core gets 256 rows
of x per batch (512 rows total), full Wq/Wo/k/v, and produces its 512 rows
of the output. No collectives, no host-side reduction.

Per-core kernel (all matmuls in float32r -> full PE rate at N>=256):
  qT[f,r]      = sum_e Wq[e,f] xT[e,r]            (PE, Wq stationary)
  simT[j,i]    = sum_d kT[d,j] qT[d,i]            (PE, kT stationary)
  es           = exp(simT * scale)                 (ACT, PSUM->SBUF)
  outT[d,i]   += v[j,d]^T es[j,i]  over j-tiles    (PE accumulate)
  s[1,i]      += ones[j,1]^T es[j,i] over j-tiles  (PE accumulate, rowsum)
  r = 1/s (DVE); rb[128,i] = ones_row^T r (PE broadcast); ACT copy to SBUF
  outn = outT * rb                                 (DVE)
  y[r,e]       = sum_f outn[f,r]^T Wo[f,e]         (PE, outn stationary)
"""

import sys
import numpy as np

for _p in ("/opt/trn_rl_repo", "/root/.axon_site/_ro/trn_rl_repo"):
    if _p not in sys.path:
        sys.path.append(_p)

import concourse.bass as bass  # noqa: E402
import concourse.mybir as mybir  # noqa: E402
import concourse.tile as tile  # noqa: E402
from concourse import bacc  # noqa: E402
from concourse.bass_utils import run_bass_kernel_spmd  # noqa: E402

F32 = mybir.dt.float32
F32R = mybir.dt.float32r

B = 2
N = 2048          # query length (global)
J = 2048          # kv length
E = 2048          # model dim
HEADS = 16
DH = 128          # head dim
NCORES = 8
NC_ROWS = N // NCORES        # 256 query rows per core per batch
R = B * NC_ROWS              # 512 rows per core, col = b*NC_ROWS + i
ET = E // 128                # 16 e-tiles
FT = HEADS                   # 16 f-tiles (one per head, DH == 128)
JT = J // 128                # 16 j-tiles
SCALE = float(DH) ** -0.5

_CACHE = {}


def _build(reps: int = 1):
    nc = bacc.Bacc(name=f"mqa_xattn_r{reps}")
    xt_d = nc.declare_dram_parameter("xt", [E, R], F32R, isOutput=False)
    kt_d = nc.declare_dram_parameter("kt", [B, DH, J], F32R, isOutput=False)
    v_d = nc.declare_dram_parameter("v", [B, J, DH], F32R, isOutput=False)
    wq_d = nc.declare_dram_parameter("wq", [E, E], F32R, isOutput=False)
    wo_d = nc.declare_dram_parameter("wo", [E, E], F32R, isOutput=False)
    oc_d = nc.declare_dram_parameter("ones_col", [128, 1], F32R, isOutput=False)
    or_d = nc.declare_dram_parameter("ones_row", [1, 128], F32R, isOutput=False)
    o_d = nc.declare_dram_parameter("o", [R, E], F32, isOutput=True)

    with tile.TileContext(nc) as tc:
        for _ in range(reps):
            _emit_once(nc, tc, xt_d, kt_d, v_d, wq_d, wo_d, oc_d, or_d, o_d)

    nc.compile()
    return nc


def _emit_once(nc, tc, xt_d, kt_d, v_d, wq_d, wo_d, oc_d, or_d, o_d):
    with tc.tile_pool(name="persist", bufs=1) as pp, \
         tc.tile_pool(name="consts", bufs=1) as cp:
        xt_sb = pp.tile([128, ET, R], F32R)
        kt_sb = pp.tile([128, B, J], F32R)
        v_sb = pp.tile([128, B, JT, DH], F32R)
        qt_all = pp.tile([128, FT, R], F32R)
        outn_all = pp.tile([128, B, FT, NC_ROWS], F32R)
        oc = cp.tile([128, 1], F32R)
        orr = cp.tile([1, 128], F32R)

        nc.sync.dma_start(oc[:], oc_d[:])
        nc.sync.dma_start(orr[:], or_d[:])
        nc.sync.dma_start(xt_sb[:], xt_d.rearrange("(et p) r -> p et r", p=128))
        nc.sync.dma_start(kt_sb[:], kt_d.rearrange("b p j -> p b j"))
        nc.sync.dma_start(v_sb[:], v_d.rearrange("b (jt p) d -> p b jt d", p=128))

        # ---- Phase B: q-projection + attention, per head ----
        with tc.tile_pool(name="wq_pool", bufs=2) as wqp, \
             tc.tile_pool(name="es_pool", bufs=3) as esp, \
             tc.tile_pool(name="rb_pool", bufs=2) as rbp, \
             tc.tile_pool(name="qp_ps", bufs=1, space="PSUM") as qp_ps, \
             tc.tile_pool(name="sg_ps", bufs=2, space="PSUM") as sg_ps, \
             tc.tile_pool(name="acc_ps", bufs=2, space="PSUM") as acc_ps, \
             tc.tile_pool(name="s_ps", bufs=1, space="PSUM") as s_psp:
            for h in range(HEADS):
                # q-projection for head h: qT[h] = Wq[:, h*128:...]^T-style
                wq_sb = wqp.tile([128, ET, 128], F32R, tag="wq")
                nc.sync.dma_start(
                    wq_sb[:],
                    wq_d[:, h * 128:(h + 1) * 128].rearrange(
                        "(et p) f -> p et f", p=128),
                )
                q_ps = qp_ps.tile([128, R], F32, tag="qp")
                for et in range(ET):
                    nc.tensor.matmul(q_ps[:], wq_sb[:, et, :], xt_sb[:, et, :],
                                     start=(et == 0), stop=(et == ET - 1))
                nc.scalar.copy(qt_all[:, h, :], q_ps[:])

                for b in range(B):
                    # NOTE: matmul start/stop accumulation groups are PSUM
                    # *bank*-granular, so outT and the rowsum need separate
                    # banks (separate tiles).
                    acc = acc_ps.tile([128, 256], F32, tag="acc")
                    s_ps = s_psp.tile([1, 256], F32, tag="s")
                    qt_h = qt_all[:, h, b * NC_ROWS:(b + 1) * NC_ROWS]
                    for jg in range(JT // 4):
                        sg = sg_ps.tile([128, 1024], F32, tag="sg")
                        for kk in range(4):
                            jt = jg * 4 + kk
                            nc.tensor.matmul(
                                sg[:, kk * 256:(kk + 1) * 256],
                                kt_sb[:, b, jt * 128:(jt + 1) * 128],
                                qt_h,
                                start=True, stop=True)
                        es = esp.tile([128, 1024], F32R, tag="es")
                        nc.scalar.activation(
                            es[:], sg[:], mybir.ActivationFunctionType.Exp,
                            scale=SCALE)
                        for kk in range(4):
                            jt = jg * 4 + kk
                            esk = es[:, kk * 256:(kk + 1) * 256]
                            nc.tensor.matmul(acc[:], v_sb[:, b, jt, :],
                                             esk, start=(jt == 0),
                                             stop=(jt == JT - 1))
                            nc.tensor.matmul(s_ps[:], oc[:], esk,
                                             start=(jt == 0),
                                             stop=(jt == JT - 1))
                    r_sb = rbp.tile([1, 256], F32R, tag="r")
                    with nc.allow_low_precision(reason="f32r==f32 bits"):
                        nc.vector.reciprocal(r_sb[:], s_ps[:])
                    # rb shares the sg pool's slots (PSUM budget: 8 banks)
                    rb_ps = sg_ps.tile([128, 256], F32, tag="sg")
                    nc.tensor.matmul(rb_ps[:], orr[:], r_sb[:],
                                     start=True, stop=True)
                    rb_sb = rbp.tile([128, 256], F32, tag="rbs")
                    nc.scalar.copy(rb_sb[:], rb_ps[:])
                    nc.vector.tensor_mul(outn_all[:, b, h, :], acc[:],
                                         rb_sb[:])

        # ---- Phase C: output projection ----
        with tc.tile_pool(name="wo_pool", bufs=8) as wop, \
             tc.tile_pool(name="ost_pool", bufs=4) as ostp, \
             tc.tile_pool(name="op_ps", bufs=4, space="PSUM") as op_ps:
            for ec in range(4):
                wo_blk = []
                for ft in range(FT):
                    wo_sb = wop.tile([128, 512], F32R, tag="wo")
                    nc.sync.dma_start(
                        wo_sb[:],
                        wo_d[ft * 128:(ft + 1) * 128,
                             ec * 512:(ec + 1) * 512])
                    wo_blk.append(wo_sb)
                for b in range(B):
                    for rt in range(2):
                        o_ps = op_ps.tile([128, 512], F32, tag="op")
                        lhs_i = slice(rt * 128, (rt + 1) * 128)
                        for ft in range(FT):
                            nc.tensor.matmul(
                                o_ps[:], outn_all[:, b, ft, lhs_i],
                                wo_blk[ft][:],
                                start=(ft == 0), stop=(ft == FT - 1))
                        o_sb = ostp.tile([128, 512], F32, tag="ost")
                        nc.vector.tensor_copy(o_sb[:], o_ps[:])
                        nc.sync.dma_start(
                            o_d[b * NC_ROWS + rt * 128:
                                b * NC_ROWS + (rt + 1) * 128,
                                ec * 512:(ec + 1) * 512],
                            o_sb[:])


def _get_nc(reps: int = 1):
    if reps not in _CACHE:
        _CACHE[reps] = _build(reps)
    return _CACHE[reps]


def _make_in_maps(x, k, v, Wq, Wo):
    kt = np.ascontiguousarray(k.transpose(0, 2, 1)).astype(np.float32)
    v_c = np.ascontiguousarray(v).astype(np.float32)
    wq = np.ascontiguousarray(Wq).astype(np.float32)
    wo = np.ascontiguousarray(Wo).astype(np.float32)
    oc = np.ones((128, 1), np.float32)
    orr = np.ones((1, 128), np.float32)
    in_maps = []
    for c in range(NCORES):
        xs = x[:, c * NC_ROWS:(c + 1) * NC_ROWS, :]
        xt = np.ascontiguousarray(
            np.concatenate([xs[0].T, xs[1].T], axis=1)).astype(np.float32)
        in_maps.append({"xt": xt, "kt": kt, "v": v_c, "wq": wq, "wo": wo,
                        "ones_col": oc, "ones_row": orr})
    return in_maps


def run_on_device(x, k, v, Wq, Wo, reps: int = 1):
    nc = _get_nc(reps)
    in_maps = _make_in_maps(x, k, v, Wq, Wo)
    res = run_bass_kernel_spmd(nc, in_maps, list(range(NCORES)))
    parts = [res.results[c]["o"].reshape(B, NC_ROWS, E) for c in range(NCORES)]
    return np.concatenate(parts, axis=1)


def kernel(x, k, v, Wq, Wo):
    x = np.asarray(x, dtype=np.float32)
    k = np.asarray(k, dtype=np.float32)
    v = np.asarray(v, dtype=np.float32)
    Wq = np.asarray(Wq, dtype=np.float32)
    Wo = np.asarray(Wo, dtype=np.float32)
    return run_on_device(x, k, v, Wq, Wo, reps=1)


# revision 7
# speedup vs baseline: 1.6973x; 1.6973x over previous
"""Trainium2 Bass kernel for MQA cross-attention (nn_CrossAttention).

Reference computation (fp32):
    q = (x @ Wq).reshape(b, n, 16, 128).transpose(0,2,1,3) * 128**-0.5
    sim = q @ k^T   (k/v shared across heads, MQA)
    out = softmax(sim) @ v
    y = out.merge_heads @ Wo

Sharding: pure sequence-parallel across 8 cores. Each core gets 256 rows
of x per batch (512 rows total), full Wq/Wo/k/v, and produces its 512 rows
of the output. No collectives, no host-side reduction.

Per-core kernel (all matmuls in float32r -> full PE rate at N>=256):
  qT[f,r]      = sum_e Wq[e,f] xT[e,r]            (PE, Wq stationary)
  simT[j,i]    = sum_d kT[d,j] qT[d,i]            (PE, kT stationary)
  es           = exp(simT * scale)                 (ACT, PSUM->SBUF)
  outT[d,i]   += v[j,d]^T es[j,i]  over j-tiles    (PE accumulate)
  s[1,i]      += ones[j,1]^T es[j,i] over j-tiles  (PE accumulate, rowsum)
  r = 1/s (DVE); rb[128,i] = ones_row^T r (PE broadcast); ACT copy to SBUF
  outn = outT * rb                                 (DVE)
  y[r,e]       = sum_f outn[f,r]^T Wo[f,e]         (PE, outn stationary)
"""

import sys
import numpy as np

for _p in ("/opt/trn_rl_repo", "/root/.axon_site/_ro/trn_rl_repo"):
    if _p not in sys.path:
        sys.path.append(_p)

import concourse.bass as bass  # noqa: E402
import concourse.mybir as mybir  # noqa: E402
import concourse.tile as tile  # noqa: E402
from concourse import bacc  # noqa: E402
from concourse.bass_utils import run_bass_kernel_spmd  # noqa: E402

F32 = mybir.dt.float32
F32R = mybir.dt.float32r

B = 2
N = 2048          # query length (global)
J = 2048          # kv length
E = 2048          # model dim
HEADS = 16
DH = 128          # head dim
NCORES = 8
NC_ROWS = N // NCORES        # 256 query rows per core per batch
R = B * NC_ROWS              # 512 rows per core, col = b*NC_ROWS + i
ET = E // 128                # 16 e-tiles
FT = HEADS                   # 16 f-tiles (one per head, DH == 128)
JT = J // 128                # 16 j-tiles
SCALE = float(DH) ** -0.5

_CACHE = {}


def _build(reps: int = 1):
    nc = bacc.Bacc(name=f"mqa_xattn_r{reps}")
    xt_d = nc.declare_dram_parameter("xt", [E, R], F32R, isOutput=False)
    kt_d = nc.declare_dram_parameter("kt", [B, DH, J], F32R, isOutput=False)
    v_d = nc.declare_dram_parameter("v", [B, J, DH], F32R, isOutput=False)
    wq_d = nc.declare_dram_parameter("wq", [E, E], F32R, isOutput=False)
    wo_d = nc.declare_dram_parameter("wo", [E, E], F32R, isOutput=False)
    oc_d = nc.declare_dram_parameter("ones_col", [128, 1], F32R, isOutput=False)
    or_d = nc.declare_dram_parameter("ones_row", [1, 128], F32R, isOutput=False)
    o_d = nc.declare_dram_parameter("o", [R, E], F32, isOutput=True)

    with tile.TileContext(nc) as tc:
        for _ in range(reps):
            _emit_once(nc, tc, xt_d, kt_d, v_d, wq_d, wo_d, oc_d, or_d, o_d)

    nc.compile()
    return nc


def _emit_once(nc, tc, xt_d, kt_d, v_d, wq_d, wo_d, oc_d, or_d, o_d):
    with tc.tile_pool(name="persist", bufs=1) as pp, \
         tc.tile_pool(name="consts", bufs=1) as cp:
        xt_sb = pp.tile([128, ET, R], F32R)
        kt_sb = pp.tile([128, B, J], F32R)
        v_sb = pp.tile([128, B, JT, DH], F32R)
        qt_all = pp.tile([128, FT, R], F32R)
        # free layout: [b][h][i] with i contiguous per head
        outn_all = pp.tile([128, B, FT * NC_ROWS], F32R)
        oc = cp.tile([128, 1], F32R)
        orr = cp.tile([1, 128], F32R)

        nc.sync.dma_start(oc[:], oc_d[:])
        nc.sync.dma_start(orr[:], or_d[:])
        nc.sync.dma_start(xt_sb[:], xt_d.rearrange("(et p) r -> p et r", p=128))
        nc.sync.dma_start(kt_sb[:], kt_d.rearrange("b p j -> p b j"))
        nc.sync.dma_start(v_sb[:], v_d.rearrange("b (jt p) d -> p b jt d", p=128))

        # ---- Phase B: q-projection + attention, per head ----
        with tc.tile_pool(name="wq_pool", bufs=2) as wqp, \
             tc.tile_pool(name="es_pool", bufs=3) as esp, \
             tc.tile_pool(name="rb_pool", bufs=2) as rbp, \
             tc.tile_pool(name="qp_ps", bufs=1, space="PSUM") as qp_ps, \
             tc.tile_pool(name="sg_ps", bufs=2, space="PSUM") as sg_ps, \
             tc.tile_pool(name="acc_ps", bufs=2, space="PSUM") as acc_ps, \
             tc.tile_pool(name="s_ps", bufs=1, space="PSUM") as s_psp:
            for hp in range(HEADS // 2):
                # q-projection for the head pair (2hp, 2hp+1)
                for hh in range(2):
                    h = 2 * hp + hh
                    wq_sb = wqp.tile([128, ET, 128], F32R, tag="wq")
                    nc.sync.dma_start(
                        wq_sb[:],
                        wq_d[:, h * 128:(h + 1) * 128].rearrange(
                            "(et p) f -> p et f", p=128),
                    )
                    q_ps = qp_ps.tile([128, R], F32, tag="qp")
                    for et in range(ET):
                        nc.tensor.matmul(q_ps[:], wq_sb[:, et, :],
                                         xt_sb[:, et, :],
                                         start=(et == 0), stop=(et == ET - 1))
                    nc.scalar.copy(qt_all[:, h, :], q_ps[:])

                for b in range(B):
                    # Both heads of the pair processed together: every matmul
                    # has a 512-wide moving operand laid out as [h2, i256].
                    # NOTE: matmul start/stop accumulation groups are PSUM
                    # *bank*-granular, so outT and the rowsum need separate
                    # banks (separate tiles).
                    acc = acc_ps.tile([128, 512], F32, tag="acc")
                    s_ps = s_psp.tile([1, 512], F32, tag="s")
                    # [128, 2, 256]: both heads' qT, this batch's rows
                    qt_pair = qt_all[:, 2 * hp:2 * hp + 2,
                                     b * NC_ROWS:(b + 1) * NC_ROWS]
                    for jg in range(JT // 2):
                        sg = sg_ps.tile([128, 1024], F32, tag="sg")
                        for kk in range(2):
                            jt = jg * 2 + kk
                            nc.tensor.matmul(
                                sg[:, kk * 512:(kk + 1) * 512],
                                kt_sb[:, b, jt * 128:(jt + 1) * 128],
                                qt_pair,
                                start=True, stop=True)
                        es = esp.tile([128, 1024], F32R, tag="es")
                        nc.scalar.activation(
                            es[:], sg[:], mybir.ActivationFunctionType.Exp,
                            scale=SCALE)
                        for kk in range(2):
                            jt = jg * 2 + kk
                            esk = es[:, kk * 512:(kk + 1) * 512]
                            nc.tensor.matmul(acc[:], v_sb[:, b, jt, :],
                                             esk, start=(jt == 0),
                                             stop=(jt == JT - 1))
                            nc.tensor.matmul(s_ps[:], oc[:], esk,
                                             start=(jt == 0),
                                             stop=(jt == JT - 1))
                    r_sb = rbp.tile([1, 512], F32R, tag="r")
                    with nc.allow_low_precision(reason="f32r==f32 bits"):
                        nc.vector.reciprocal(r_sb[:], s_ps[:])
                    # rb shares the sg pool's slots (PSUM budget: 8 banks)
                    rb_ps = sg_ps.tile([128, 512], F32, tag="sg")
                    nc.tensor.matmul(rb_ps[:], orr[:], r_sb[:],
                                     start=True, stop=True)
                    rb_sb = rbp.tile([128, 512], F32, tag="rbs")
                    nc.scalar.copy(rb_sb[:], rb_ps[:])
                    nc.vector.tensor_mul(
                        outn_all[:, b, 2 * hp * NC_ROWS:
                                 (2 * hp + 2) * NC_ROWS],
                        acc[:], rb_sb[:])

        # ---- Phase C: output projection ----
        with tc.tile_pool(name="wo_pool", bufs=8) as wop, \
             tc.tile_pool(name="ost_pool", bufs=4) as ostp, \
             tc.tile_pool(name="op_ps", bufs=4, space="PSUM") as op_ps:
            for ec in range(4):
                wo_blk = []
                for ft in range(FT):
                    wo_sb = wop.tile([128, 512], F32R, tag="wo")
                    nc.sync.dma_start(
                        wo_sb[:],
                        wo_d[ft * 128:(ft + 1) * 128,
                             ec * 512:(ec + 1) * 512])
                    wo_blk.append(wo_sb)
                for b in range(B):
                    for rt in range(2):
                        o_ps = op_ps.tile([128, 512], F32, tag="op")
                        for ft in range(FT):
                            i0 = ft * NC_ROWS + rt * 128
                            nc.tensor.matmul(
                                o_ps[:], outn_all[:, b, i0:i0 + 128],
                                wo_blk[ft][:],
                                start=(ft == 0), stop=(ft == FT - 1))
                        o_sb = ostp.tile([128, 512], F32, tag="ost")
                        nc.vector.tensor_copy(o_sb[:], o_ps[:])
                        nc.sync.dma_start(
                            o_d[b * NC_ROWS + rt * 128:
                                b * NC_ROWS + (rt + 1) * 128,
                                ec * 512:(ec + 1) * 512],
                            o_sb[:])


def _get_nc(reps: int = 1):
    if reps not in _CACHE:
        _CACHE[reps] = _build(reps)
    return _CACHE[reps]


def _make_in_maps(x, k, v, Wq, Wo):
    kt = np.ascontiguousarray(k.transpose(0, 2, 1)).astype(np.float32)
    v_c = np.ascontiguousarray(v).astype(np.float32)
    wq = np.ascontiguousarray(Wq).astype(np.float32)
    wo = np.ascontiguousarray(Wo).astype(np.float32)
    oc = np.ones((128, 1), np.float32)
    orr = np.ones((1, 128), np.float32)
    in_maps = []
    for c in range(NCORES):
        xs = x[:, c * NC_ROWS:(c + 1) * NC_ROWS, :]
        xt = np.ascontiguousarray(
            np.concatenate([xs[0].T, xs[1].T], axis=1)).astype(np.float32)
        in_maps.append({"xt": xt, "kt": kt, "v": v_c, "wq": wq, "wo": wo,
                        "ones_col": oc, "ones_row": orr})
    return in_maps


def run_on_device(x, k, v, Wq, Wo, reps: int = 1):
    nc = _get_nc(reps)
    in_maps = _make_in_maps(x, k, v, Wq, Wo)
    res = run_bass_kernel_spmd(nc, in_maps, list(range(NCORES)))
    parts = [res.results[c]["o"].reshape(B, NC_ROWS, E) for c in range(NCORES)]
    return np.concatenate(parts, axis=1)


def kernel(x, k, v, Wq, Wo):
    x = np.asarray(x, dtype=np.float32)
    k = np.asarray(k, dtype=np.float32)
    v = np.asarray(v, dtype=np.float32)
    Wq = np.asarray(Wq, dtype=np.float32)
    Wo = np.asarray(Wo, dtype=np.float32)
    return run_on_device(x, k, v, Wq, Wo, reps=1)


# revision 27
# speedup vs baseline: 1405.4796x; 828.0791x over previous
"""Trainium2 Bass kernel for MQA cross-attention (nn_CrossAttention).

Reference computation (fp32):
    q = (x @ Wq).reshape(b, n, 16, 128).transpose(0,2,1,3) * 128**-0.5
    sim = q @ k^T   (k/v shared across heads, MQA)
    out = softmax(sim) @ v
    y = out.merge_heads @ Wo

Sharding: pure sequence-parallel across 8 cores. Each core gets 256 rows
of x per batch (512 rows total), full Wq/Wo/k/v, and produces its 512 rows
of the output. No collectives, no host-side reduction.

Per-core kernel (all matmuls in float32r -> full PE rate at N>=256):
  qT[f,r]      = sum_e Wq[e,f] xT[e,r]            (PE, Wq stationary)
  simT[j,i]    = sum_d kT[d,j] qT[d,i]            (PE, kT stationary)
  es           = exp(simT * scale)                 (ACT, PSUM->SBUF)
  outT[d,i]   += v[j,d]^T es[j,i]  over j-tiles    (PE accumulate)
  s[1,i]      += ones[j,1]^T es[j,i] over j-tiles  (PE accumulate, rowsum)
  r = 1/s (DVE); rb[128,i] = ones_row^T r (PE broadcast); ACT copy to SBUF
  outn = outT * rb                                 (DVE)
  y[r,e]       = sum_f outn[f,r]^T Wo[f,e]         (PE, outn stationary)
"""

import sys
import numpy as np

for _p in ("/opt/trn_rl_repo", "/root/.axon_site/_ro/trn_rl_repo"):
    if _p not in sys.path:
        sys.path.append(_p)

import concourse.bass as bass  # noqa: E402
import concourse.mybir as mybir  # noqa: E402
import concourse.tile as tile  # noqa: E402
from concourse import bacc, bass_isa  # noqa: E402
from concourse.bass_utils import run_bass_kernel_spmd  # noqa: E402

F32 = mybir.dt.float32
F32R = mybir.dt.float32r

B = 2
N = 2048          # query length (global)
J = 2048          # kv length
E = 2048          # model dim
HEADS = 16
DH = 128          # head dim
NCORES = 8
NC_ROWS = N // NCORES        # 256 query rows per core per batch
R = B * NC_ROWS              # 512 rows per core, col = b*NC_ROWS + i
ET = E // 128                # 16 e-tiles
FT = HEADS                   # 16 f-tiles (one per head, DH == 128)
JT = J // 128                # 16 j-tiles
SCALE = float(DH) ** -0.5

_CACHE = {}


def _build(reps: int = 1):
    nc = bacc.Bacc(name=f"mqa_xattn_r{reps}")
    xt_d = nc.declare_dram_parameter("xt", [E, R], F32R, isOutput=False)
    kt_d = nc.declare_dram_parameter("kt", [B, DH, J], F32R, isOutput=False)
    v_d = nc.declare_dram_parameter("v", [B, J, DH], F32R, isOutput=False)
    wq_d = nc.declare_dram_parameter("wq", [E, E], F32R, isOutput=False)
    wo_d = nc.declare_dram_parameter("wo", [E, E], F32R, isOutput=False)
    o_d = nc.declare_dram_parameter("o", [R, E], F32, isOutput=True)

    with tile.TileContext(nc) as tc:
        for _ in range(reps):
            _emit_once(nc, tc, xt_d, kt_d, v_d, wq_d, wo_d, o_d)

    nc.compile()
    return nc


def _emit_once(nc, tc, xt_d, kt_d, v_d, wq_d, wo_d, o_d):
    with tc.tile_pool(name="persist", bufs=1) as pp:
        kt_sb = pp.tile([128, B, J], F32R)
        v_sb = pp.tile([128, B, JT, DH], F32R)
        qt_all = pp.tile([128, FT, R], F32R)
        # free layout: [b][h][i] with i contiguous per head
        outn_all = pp.tile([128, B, FT * NC_ROWS], F32R)

        # ---- Phase B: q-projection + attention, per head ----
        # xt lives in its own pool, released before phase C so its SBUF
        # space can hold the Wo prefetch.
        with tc.tile_pool(name="xt_pool", bufs=1) as xtp, \
             tc.tile_pool(name="wq_pool", bufs=3) as wqp, \
             tc.tile_pool(name="es_pool", bufs=4) as esp, \
             tc.tile_pool(name="rb_pool", bufs=2) as rbp, \
             tc.tile_pool(name="qp_ps", bufs=1, space="PSUM") as qp_ps, \
             tc.tile_pool(name="sg_ps", bufs=2, space="PSUM") as sg_ps, \
             tc.tile_pool(name="acc_ps", bufs=3, space="PSUM") as acc_ps:
            xt_sb = xtp.tile([128, ET, R], F32R)

            def load_wq(h):
                wq_sb = wqp.tile([128, ET, 128], F32R, tag="wq",
                                 name=f"wq_sb{h}")
                nc.sync.dma_start(
                    wq_sb[:],
                    wq_d[:, h * 128:(h + 1) * 128].rearrange(
                        "(et p) f -> p et f", p=128),
                )
                return wq_sb

            # DMA order: head-0 Wq and x interleaved in fine chunks so the
            # first qproj matmuls start as early as possible, then k/v in
            # batch order (attention consumes batch 0 first).
            wq_next = wqp.tile([128, ET, 128], F32R, tag="wq", name="wq_sb0")
            wq0_r = wq_d[:, 0:128].rearrange("(et p) f -> p et f", p=128)
            xt_r = xt_d.rearrange("(et p) r -> p et r", p=128)
            for c in range(4):
                nc.sync.dma_start(wq_next[:, 4 * c:4 * (c + 1), :],
                                  wq0_r[:, 4 * c:4 * (c + 1), :])
                nc.sync.dma_start(xt_sb[:, 4 * c:4 * (c + 1), :],
                                  xt_r[:, 4 * c:4 * (c + 1), :])
            wq_next2 = load_wq(1)
            kt_r = kt_d.rearrange("b p j -> p b j")
            v_r = v_d.rearrange("b (jt p) d -> p b jt d", p=128)
            for b in range(B):
                nc.sync.dma_start(kt_sb[:, b, :], kt_r[:, b, :])
                nc.sync.dma_start(v_sb[:, b, :, :], v_r[:, b, :, :])

            def qproj_pair(hp):
                nonlocal wq_next, wq_next2
                for hh in range(2):
                    h = 2 * hp + hh
                    wq_sb = wq_next
                    wq_next = wq_next2
                    if h + 2 < HEADS:
                        wq_next2 = load_wq(h + 2)
                    q_ps = qp_ps.tile([128, R], F32, tag="qp")
                    for et in range(ET):
                        nc.tensor.matmul(q_ps[:], wq_sb[:, et, :],
                                         xt_sb[:, et, :],
                                         start=(et == 0), stop=(et == ET - 1))
                    nc.scalar.copy(qt_all[:, h, :], q_ps[:])

            # pair hp's q-projection is emitted during pair hp-1's first
            # attention unit, so its ACT copies land in ACT slack and qT is
            # ready before pair hp's simT needs it.
            qproj_pair(0)
            for hp in range(HEADS // 2):
                for b in range(B):
                    if b == 1 and hp + 1 < HEADS // 2:
                        qproj_pair(hp + 1)
                    # Both heads of the pair processed together: every matmul
                    # has a 512-wide moving operand laid out as [h2, i256].
                    # NOTE: matmul start/stop accumulation groups are PSUM
                    # *bank*-granular, so outT and the rowsum need separate
                    # banks (separate tiles).
                    acc = acc_ps.tile([128, 512], F32, tag="acc")
                    # [128, 2, 256]: both heads' qT, this batch's rows
                    qt_pair = qt_all[:, 2 * hp:2 * hp + 2,
                                     b * NC_ROWS:(b + 1) * NC_ROWS]
                    s1024 = rbp.tile([128, 1024], F32R, tag="s128")
                    for jg in range(JT // 2):
                        sg = sg_ps.tile([128, 1024], F32, tag="sg")
                        for kk in range(2):
                            jt = jg * 2 + kk
                            nc.tensor.matmul(
                                sg[:, kk * 512:(kk + 1) * 512],
                                kt_sb[:, b, jt * 128:(jt + 1) * 128],
                                qt_pair,
                                start=True, stop=True)
                        es = esp.tile([128, 1024], F32R, tag="es")
                        nc.scalar.activation(
                            es[:], sg[:], mybir.ActivationFunctionType.Exp,
                            scale=SCALE)
                        # softmax denominators: partial row-sums on DVE
                        # (j-partition partials; the 128-way partition
                        # reduction is one ones-matmul below)
                        with nc.allow_low_precision(reason="f32r==f32 bits"):
                            if jg == 0:
                                nc.vector.tensor_copy(s1024[:], es[:])
                            else:
                                nc.vector.tensor_add(s1024[:], s1024[:], es[:])
                        for kk in range(2):
                            jt = jg * 2 + kk
                            esk = es[:, kk * 512:(kk + 1) * 512]
                            nc.tensor.matmul(acc[:], v_sb[:, b, jt, :],
                                             esk, start=(jt == 0),
                                             stop=(jt == JT - 1))
                    # softmax-denominator tail: entirely off the PE stream
                    # (DVE fold -> gpsimd partition all-reduce -> DVE recip
                    #  -> DVE normalize)
                    s512 = rbp.tile([128, 512], F32R, tag="s512", bufs=1)
                    sB = rbp.tile([128, 512], F32R, tag="sB", bufs=1)
                    rb_sb = rbp.tile([128, 512], F32R, tag="rbs")
                    with nc.allow_low_precision(reason="f32r==f32 bits"):
                        nc.vector.tensor_add(s512[:], s1024[:, 0:512],
                                             s1024[:, 512:1024])
                        nc.gpsimd.partition_all_reduce(
                            sB[:], s512[:], channels=128,
                            reduce_op=bass_isa.ReduceOp.add)
                        nc.vector.reciprocal(rb_sb[:], sB[:])
                    nc.vector.tensor_mul(
                        outn_all[:, b, 2 * hp * NC_ROWS:
                                 (2 * hp + 2) * NC_ROWS],
                        acc[:], rb_sb[:])

        # ---- Phase C: output projection ----
        # Per (ec, ft): one Wo block DMA feeding 4 accumulating matmuls;
        # wo_pool depth lets the Wo stream prefetch during late attention.
        with tc.tile_pool(name="wo_pool", bufs=24) as wop, \
             tc.tile_pool(name="ost_pool", bufs=4) as ostp, \
             tc.tile_pool(name="op_ps", bufs=4, space="PSUM") as op_ps:
            for ec in range(4):
                wo_blk = []
                for ft in range(FT):
                    wo_sb = wop.tile([128, 512], F32R, tag="wo")
                    nc.sync.dma_start(
                        wo_sb[:],
                        wo_d[ft * 128:(ft + 1) * 128,
                             ec * 512:(ec + 1) * 512])
                    wo_blk.append(wo_sb)
                for b in range(B):
                    for rt in range(2):
                        o_ps = op_ps.tile([128, 512], F32, tag="op")
                        for ft in range(FT):
                            i0 = ft * NC_ROWS + rt * 128
                            nc.tensor.matmul(
                                o_ps[:], outn_all[:, b, i0:i0 + 128],
                                wo_blk[ft][:],
                                start=(ft == 0), stop=(ft == FT - 1))
                        o_sb = ostp.tile([128, 512], F32, tag="ost")
                        nc.vector.tensor_copy(o_sb[:], o_ps[:])
                        nc.sync.dma_start(
                            o_d[b * NC_ROWS + rt * 128:
                                b * NC_ROWS + (rt + 1) * 128,
                                ec * 512:(ec + 1) * 512],
                            o_sb[:])


def _get_nc(reps: int = 1):
    if reps not in _CACHE:
        _CACHE[reps] = _build(reps)
    return _CACHE[reps]


def _make_in_maps(x, k, v, Wq, Wo):
    kt = np.ascontiguousarray(k.transpose(0, 2, 1)).astype(np.float32)
    v_c = np.ascontiguousarray(v).astype(np.float32)
    wq = np.ascontiguousarray(Wq).astype(np.float32)
    wo = np.ascontiguousarray(Wo).astype(np.float32)
    in_maps = []
    for c in range(NCORES):
        xs = x[:, c * NC_ROWS:(c + 1) * NC_ROWS, :]
        xt = np.ascontiguousarray(
            np.concatenate([xs[0].T, xs[1].T], axis=1)).astype(np.float32)
        in_maps.append({"xt": xt, "kt": kt, "v": v_c, "wq": wq, "wo": wo})
    return in_maps


def run_on_device(x, k, v, Wq, Wo, reps: int = 1):
    nc = _get_nc(reps)
    in_maps = _make_in_maps(x, k, v, Wq, Wo)
    res = run_bass_kernel_spmd(nc, in_maps, list(range(NCORES)))
    parts = [res.results[c]["o"].reshape(B, NC_ROWS, E) for c in range(NCORES)]
    return np.concatenate(parts, axis=1)


def kernel(x, k, v, Wq, Wo):
    x = np.asarray(x, dtype=np.float32)
    k = np.asarray(k, dtype=np.float32)
    v = np.asarray(v, dtype=np.float32)
    Wq = np.asarray(Wq, dtype=np.float32)
    Wo = np.asarray(Wo, dtype=np.float32)
    return run_on_device(x, k, v, Wq, Wo, reps=1)
